# revision 6
# baseline (speedup 1.0000x reference)
"""Trainium2 Bass kernel for InterpretableMultiHeadAttention.

Shapes (hardcoded): B=2, S=2048, D=1024, H=16, DK=64.
Reference math:
    qs = einsum("bsd,hdk->bhsk", q, Wq); ks likewise with Wk
    vs = v @ Wv                       (shared across heads)
    attn = softmax(qs @ ks^T / 8)     [B,H,S,S]
    attn_mean = attn.mean(h)          [B,S,S]   (output 2)
    head = heads.mean(h) = attn_mean @ vs       (Wv shared => exact)
    out = head @ Wo                   [B,S,D]   (output 1)

Sharding: 8 cores = (batch, 512-query chunk). Each core owns 512 query rows
of one batch and computes its slice of both outputs; host does a pure gather.

Device-side per core (all matmuls bf16, accumulation fp32 in PSUM):
  - qsT = wq^T qT; ksT m-tiles interleaved into the first s-tile's head loop
    so ScalarE exp work starts early; vsT = wv^T vT then PE-transpose to vs.
  - per (s-tile of 128 queries, head): scores(psum) = qsT_h^T ksT_h;
    exp on ScalarE (scale=1/8) with fused row-sum; reciprocal ->
    diag(1/sum)/16; PE matmul-accumulates E = sum_h diag_h @ exp_h
    (= attn_mean rows) in PSUM across all 16 heads.
  - tail per s-tile: E -> sbuf (f32 for DMA, bf16 copy), PE-transpose E,
    headT = vs^T @ E^T; finally out rows = headT^T @ Wo.
"""

import numpy as np
import ml_dtypes
from contextlib import ExitStack

import concourse.bass as bass
import concourse.tile as tile
from concourse import bacc, mybir
from concourse.bass_utils import run_bass_kernel_spmd
from concourse.masks import make_identity

B, S, D, H, DK = 2, 2048, 1024, 16, 64
P = 128
SQ = 512                 # query rows per core
NDC = D // P             # 8 d-chunks
NMT = D // P             # 8 m-tiles over (h,dk)
NST = SQ // P            # 4 s-tiles per core
NTC = S // P             # 16 t-chunks
N_CORES = 8

BF = mybir.dt.bfloat16
F32 = mybir.dt.float32
EXPF = mybir.ActivationFunctionType.Exp

_compiled = {}


def _build_program():
    nc = bacc.Bacc("TRN2", target_bir_lowering=False, debug=False,
                   num_devices=N_CORES)

    qT = nc.dram_tensor("qT", [D, SQ], BF, kind="ExternalInput").ap()
    kT = nc.dram_tensor("kT", [D, S], BF, kind="ExternalInput").ap()
    vT = nc.dram_tensor("vT", [D, S], BF, kind="ExternalInput").ap()
    wq = nc.dram_tensor("wq", [D, D], BF, kind="ExternalInput").ap()
    wk = nc.dram_tensor("wk", [D, D], BF, kind="ExternalInput").ap()
    wv = nc.dram_tensor("wv", [D, DK], BF, kind="ExternalInput").ap()
    wo = nc.dram_tensor("wo", [DK, D], BF, kind="ExternalInput").ap()
    out_part = nc.dram_tensor("out_part", [SQ, D], F32, kind="ExternalOutput").ap()
    am_part = nc.dram_tensor("am_part", [SQ, S], F32, kind="ExternalOutput").ap()

    with tile.TileContext(nc) as tc, ExitStack() as ctx:
        pers = ctx.enter_context(tc.tile_pool(name="pers", bufs=1))

        ident = pers.tile([P, P], BF, tag="ident")
        make_identity(nc, ident)
        # identity pre-scaled by 1/H for the attn-mean accumulation
        id16 = pers.tile([P, P], BF, tag="id16")
        nc.gpsimd.memset(id16, 0.0)
        nc.gpsimd.affine_select(
            out=id16, in_=id16, compare_op=mybir.AluOpType.not_equal,
            fill=1.0 / H, base=0, pattern=[[-1, P]], channel_multiplier=1)

        # ---- persistent SBUF arrays ------------------------------------
        qT_sb = [pers.tile([P, SQ], BF, tag=f"qT{i}", name=f"qT_sb{i}")
                 for i in range(NDC)]
        kT_sb = [pers.tile([P, S], BF, tag=f"kT{i}", name=f"kT_sb{i}")
                 for i in range(NDC)]
        vT_sb = [pers.tile([P, S], BF, tag=f"vT{i}", name=f"vT_sb{i}")
                 for i in range(NDC)]
        wq_sb = [pers.tile([P, D], BF, tag=f"wq{i}", name=f"wq_sb{i}")
                 for i in range(NDC)]
        wk_sb = [pers.tile([P, D], BF, tag=f"wk{i}", name=f"wk_sb{i}")
                 for i in range(NDC)]
        wv_sb = [pers.tile([P, DK], BF, tag=f"wv{i}", name=f"wv_sb{i}")
                 for i in range(NDC)]
        wo_sb = pers.tile([DK, D], BF, tag="wo")
        qsT_sb = [pers.tile([P, SQ], BF, tag=f"qsT{m}", name=f"qsT_sb{m}")
                  for m in range(NMT)]
        ksT_sb = [pers.tile([P, S], BF, tag=f"ksT{m}", name=f"ksT_sb{m}")
                  for m in range(NMT)]
        vsT_sb = pers.tile([DK, S], BF, tag="vsT")
        vs_sb = [pers.tile([P, DK], BF, tag=f"vs{t}", name=f"vs_sb{t}")
                 for t in range(NTC)]
        headT_sb = pers.tile([DK, SQ], BF, tag="headT")

        # ---- input DMAs (ordered by first use) -------------------------
        for i in range(NDC):
            r = slice(i * P, (i + 1) * P)
            nc.sync.dma_start(qT_sb[i][:], qT[r, :])
            nc.sync.dma_start(wq_sb[i][:], wq[r, :])
        for i in range(NDC):
            r = slice(i * P, (i + 1) * P)
            nc.sync.dma_start(wk_sb[i][:], wk[r, :])
            nc.sync.dma_start(kT_sb[i][:], kT[r, :])
        for i in range(NDC):
            r = slice(i * P, (i + 1) * P)
            nc.sync.dma_start(wv_sb[i][:], wv[r, :])
            nc.sync.dma_start(vT_sb[i][:], vT[r, :])
        nc.sync.dma_start(wo_sb[:], wo[:, :])

        psE = ctx.enter_context(tc.tile_pool(name="psE", bufs=1, space="PSUM"))
        psSc = ctx.enter_context(tc.tile_pool(name="psSc", bufs=2, space="PSUM"))
        wrk = ctx.enter_context(tc.tile_pool(name="wrk", bufs=2))
        sm = ctx.enter_context(tc.tile_pool(name="sm", bufs=3))

        # ---- qsT projection: qsT[m] = (wq m-tile)^T @ qT ---------------
        for m in range(NMT):
            mc = slice(m * P, (m + 1) * P)
            ps = psSc.tile([P, 1024], F32, tag="sc", name=f"psq{m}")
            for d in range(NDC):
                nc.tensor.matmul(ps[:, 0:SQ], wq_sb[d][:, mc],
                                 qT_sb[d][:], start=(d == 0),
                                 stop=(d == NDC - 1))
            nc.vector.tensor_copy(qsT_sb[m][:], ps[:, 0:SQ])

        def emit_ksT(m):
            mc = slice(m * P, (m + 1) * P)
            for j in range(2):
                ps = psSc.tile([P, 1024], F32, tag="sc", name=f"psk{m}_{j}")
                for jj in range(2):
                    cs = slice(j * 1024 + jj * 512, j * 1024 + (jj + 1) * 512)
                    for d in range(NDC):
                        nc.tensor.matmul(ps[:, jj * 512:(jj + 1) * 512],
                                         wk_sb[d][:, mc], kT_sb[d][:, cs],
                                         start=(d == 0), stop=(d == NDC - 1))
                nc.vector.tensor_copy(
                    ksT_sb[m][:, j * 1024:(j + 1) * 1024], ps[:])

        def emit_vs():
            # vsT = wv^T @ vT, then vs[t] = transpose(vsT t-chunk)
            for j in range(4):
                cs = slice(j * 512, (j + 1) * 512)
                ps = psSc.tile([DK, 1024], F32, tag="sc", name=f"psv{j}")
                for d in range(NDC):
                    nc.tensor.matmul(ps[:, 0:512], wv_sb[d][:],
                                     vT_sb[d][:, cs],
                                     start=(d == 0), stop=(d == NDC - 1))
                nc.vector.tensor_copy(vsT_sb[:, cs], ps[:, 0:512])
            for t in range(NTC):
                tc_ = slice(t * P, (t + 1) * P)
                ps = psSc.tile([P, DK], BF, tag="sc", name=f"psvt{t}")
                nc.tensor.transpose(ps[:], vsT_sb[:, tc_],
                                    ident[0:DK, 0:DK])
                nc.vector.tensor_copy(vs_sb[t][:], ps[:])

        # ---- attention: per s-tile, accumulate E over heads ------------
        for st in range(NST):
            sc_ = slice(st * P, (st + 1) * P)
            E = psE.tile([P, S], F32, tag="E", name=f"E{st}")
            for h in range(H):
                m, r0 = h // 2, (h % 2) * DK
                rr = slice(r0, r0 + DK)
                if st == 0 and h % 2 == 0:
                    emit_ksT(m)          # ksT m-tile just before first use
                exp_sb = wrk.tile([P, S], BF, tag="exp", bufs=3,
                                  name=f"exp{st}_{h}")
                rs_parts = []
                for j in range(2):
                    ps = psSc.tile([P, 1024], F32, tag="sc",
                                   name=f"sc{st}_{h}_{j}")
                    for jj in range(2):
                        cs = slice(j * 1024 + jj * 512,
                                   j * 1024 + (jj + 1) * 512)
                        nc.tensor.matmul(ps[:, jj * 512:(jj + 1) * 512],
                                         qsT_sb[m][rr, sc_],
                                         ksT_sb[m][rr, cs],
                                         start=True, stop=True)
                    rs = sm.tile([P, 1], F32, tag=f"rs{j}",
                                 name=f"rs{st}_{h}_{j}")
                    nc.scalar.activation(
                        exp_sb[:, j * 1024:(j + 1) * 1024], ps[:],
                        EXPF, scale=0.125, accum_out=rs[:])
                    rs_parts.append(rs)
                rtot = sm.tile([P, 1], F32, tag="rtot", name=f"rt{st}_{h}")
                nc.vector.tensor_add(rtot[:], rs_parts[0][:], rs_parts[1][:])
                rcp = sm.tile([P, 1], F32, tag="rcp", name=f"rcp{st}_{h}")
                nc.vector.reciprocal(rcp[:], rtot[:])
                dg = sm.tile([P, P], BF, tag="diag", bufs=4,
                             name=f"dg{st}_{h}")
                nc.vector.tensor_scalar_mul(dg[:], id16[:], rcp[:])
                for j in range(S // 512):
                    cs = slice(j * 512, (j + 1) * 512)
                    nc.tensor.matmul(E[:, cs], dg[:], exp_sb[:, cs],
                                     start=(h == 0), stop=(h == H - 1))

            if st == 0:
                emit_vs()

            # ---- tail: E holds attn_mean rows for this s-tile ----------
            am_sb = wrk.tile([P, S], F32, tag="am", bufs=1, name=f"am{st}")
            nc.vector.tensor_copy(am_sb[:], E[:])
            nc.sync.dma_start(am_part[sc_, :], am_sb[:])
            Ebf = wrk.tile([P, S], BF, tag="Ebf", bufs=1, name=f"Ebf{st}")
            nc.scalar.copy(Ebf[:], E[:])

            # E^T via PE transpose (rotates through E's psum banks)
            et_ps = psE.tile([P, S], BF, tag="E", name=f"et{st}")
            for t in range(NTC):
                tc_ = slice(t * P, (t + 1) * P)
                nc.tensor.transpose(et_ps[:, tc_], Ebf[:, tc_], ident[:])
            ET_sb = wrk.tile([P, S], BF, tag="ET", bufs=1, name=f"ET{st}")
            nc.vector.tensor_copy(ET_sb[:], et_ps[:])

            # headT[:, s-tile] = sum_t vs[t]^T @ ET[t]
            hps = psE.tile([DK, P], F32, tag="E", name=f"hps{st}")
            for t in range(NTC):
                tc_ = slice(t * P, (t + 1) * P)
                nc.tensor.matmul(hps[:], vs_sb[t][:], ET_sb[:, tc_],
                                 start=(t == 0), stop=(t == NTC - 1))
            nc.vector.tensor_copy(headT_sb[:, sc_], hps[:])

            # ---- output projection for this s-tile ---------------------
            ops = psSc.tile([P, 1024], F32, tag="sc", name=f"ops{st}")
            for j in range(2):
                cs = slice(j * 512, (j + 1) * 512)
                nc.tensor.matmul(ops[:, cs], headT_sb[:, sc_],
                                 wo_sb[:, cs], start=True, stop=True)
            o_sb = wrk.tile([P, 1024], F32, tag="osb", name=f"osb{st}")
            nc.vector.tensor_copy(o_sb[:], ops[:])
            nc.sync.dma_start(out_part[sc_, :], o_sb[:])

    nc.compile()
    return nc


def _get_program():
    if "nc" not in _compiled:
        _compiled["nc"] = _build_program()
    return _compiled["nc"]


def _make_in_maps(q, k, v, Wq, Wk, Wv, Wo):
    bf = ml_dtypes.bfloat16
    wq_r = np.ascontiguousarray(
        Wq.transpose(1, 0, 2).reshape(D, D)).astype(bf)
    wk_r = np.ascontiguousarray(
        Wk.transpose(1, 0, 2).reshape(D, D)).astype(bf)
    wv_r = np.ascontiguousarray(Wv).astype(bf)
    wo_r = np.ascontiguousarray(Wo).astype(bf)
    kT = [np.ascontiguousarray(k[b].T).astype(bf) for b in range(B)]
    vT = [np.ascontiguousarray(v[b].T).astype(bf) for b in range(B)]
    in_maps = []
    for c in range(N_CORES):
        b, j = c // 4, c % 4
        qT_c = np.ascontiguousarray(
            q[b, j * SQ:(j + 1) * SQ, :].T).astype(bf)
        in_maps.append({
            "qT": qT_c, "kT": kT[b], "vT": vT[b],
            "wq": wq_r, "wk": wk_r, "wv": wv_r, "wo": wo_r,
        })
    return in_maps


def run_sharded(q, k, v, Wq, Wk, Wv, Wo, trace=False):
    nc = _get_program()
    in_maps = _make_in_maps(q, k, v, Wq, Wk, Wv, Wo)
    res = run_bass_kernel_spmd(nc, in_maps, list(range(N_CORES)), trace=trace)
    out = np.empty((B, S, D), np.float32)
    am = np.empty((B, S, S), np.float32)
    for c in range(N_CORES):
        b, j = c // 4, c % 4
        sl = slice(j * SQ, (j + 1) * SQ)
        out[b, sl] = res.results[c]["out_part"]
        am[b, sl] = res.results[c]["am_part"]
    return (out, am), res


def kernel(q, k, v, Wq, Wk, Wv, Wo):
    (out, am), _ = run_sharded(q, k, v, Wq, Wk, Wv, Wo)
    return (out, am)
